# revision 37
# baseline (speedup 1.0000x reference)
"""Trainium2 Bass kernel for nn_Attention_13073880449373.

Full-batch multi-head attention (B=8, S=1024, C=1024, H=16, D=64) with RoPE,
data-parallel over the batch dim: core b computes batch b end-to-end.

v2 design (all matmul operands bf16, fp32 PSUM accumulation, no DRAM staging):
  xT (C,S) bf16 --[Wqk pair chunks stationary]--> qkT (128,S) psum, per-pair
  DVE bias-add (per-partition) -> bf16, RoPE (rotate-half SBUF DMA + 2x TT)
  scoresT (sk,sq) = kT.T @ qT per head, both heads of a pair row-paired on
  disjoint PE row groups; ONE exp (ACT, FD=1024 spanning both heads' PSUM
  banks) -> pT bf16
  outT (65,sq) accumulates [v|1].T @ pT per (head, n-half); row 64 = softmax
  denominators.  v lives in SBUF (vst tiles, ones column interleaved) - PV
  stationary slices read it directly.
  Denominator rows are copied out per n-half (frees PSUM fast), reciprocals
  batched as one (4,512) DVE op per pair, GPSIMD partition-broadcast, one
  (128,1024) TT multiply normalizes the pair -> aT bf16 in SBUF.
  out (S,C) f32 = aT.T @ Wp + bias (bias via broadcast tile + TT add).
  v-phase matmuls interleave into pair 0's score steps; pair p+1's qk chunks
  interleave into pair p's steps, keeping PE dense (HAM stays at 2.4 GHz).
"""

import math
import os
from contextlib import ExitStack

import numpy as np

B, S, C = 8, 1024, 1024
H, D = 16, 64
N_CORES = 8
KC = C // 128  # 8 contraction chunks of 128

_CACHE = {}


def _cs_table():
    # Matches reference.rope_cos_sin computed in float32, transposed, with the
    # rotate-half sign folded into the sin half (rows 0-31 negated).
    f = np.float32
    inv = np.exp(np.arange(0, D, 2, dtype=f) * f(-(math.log(10000.0) / D))).astype(f)
    pos = np.arange(S, dtype=f)[:, None]
    ang = (pos * inv[None, :]).astype(f)  # (S, 32)
    ang = np.concatenate([ang, ang], axis=1)  # (S, 64)
    cosT = np.cos(ang).T.astype(f)  # (64, S)
    sinT = np.sin(ang).T.astype(f)
    sign = np.where(np.arange(D) < D // 2, f(-1.0), f(1.0))[:, None].astype(f)
    half = np.concatenate([cosT, sinT * sign], axis=1)  # (64, 2S)
    return np.concatenate([half, half], axis=0).astype(f)  # (128, 2S)


def declare_io(nc):
    from concourse import mybir

    f32 = mybir.dt.float32
    bf16 = mybir.dt.bfloat16
    return {
        "xT": nc.dram_tensor("xT", [C, S], bf16, kind="ExternalInput").ap(),
        "Wqk": nc.dram_tensor("Wqk", [C, 2 * C], bf16, kind="ExternalInput").ap(),
        "bqk": nc.dram_tensor("bqk", [128, 16], f32, kind="ExternalInput").ap(),
        "Wv": nc.dram_tensor("Wv", [C, C], bf16, kind="ExternalInput").ap(),
        "bv": nc.dram_tensor("bv", [1, C], f32, kind="ExternalInput").ap(),
        "Wp": nc.dram_tensor("Wp", [C, C], bf16, kind="ExternalInput").ap(),
        "bp": nc.dram_tensor("bp", [1, C], f32, kind="ExternalInput").ap(),
        "cs": nc.dram_tensor("cs", [128, 2 * S], bf16, kind="ExternalInput").ap(),
        "out": nc.dram_tensor("out", [S, C], f32, kind="ExternalOutput").ap(),
    }


def _emit(tc, io=None):
    from concourse import mybir
    from concourse.bass import ds, ts

    nc = tc.nc
    f32 = mybir.dt.float32
    bf16 = mybir.dt.bfloat16
    AF = mybir.ActivationFunctionType
    MUL = mybir.AluOpType.mult
    ADD = mybir.AluOpType.add

    if io is None:
        io = declare_io(nc)
    xT, Wqk, bqk, Wv, bv, Wp, bp, cs, out = (
        io[k] for k in ["xT", "Wqk", "bqk", "Wv", "bv", "Wp", "bp", "cs", "out"]
    )

    with ExitStack() as ctx:
        # ---------------- long-lived consts ----------------
        kons = ctx.enter_context(tc.tile_pool(name="kons", bufs=1, side="right"))
        cs_t = kons.tile([128, 2 * S], bf16, name="cs_t")
        bqk_t = kons.tile([128, 16], f32, name="bqk_t")
        bv_sb = kons.tile([1, C], f32, name="bv_sb")
        bp_sb = kons.tile([1, C], f32, name="bp_sb")
        bv_bc = kons.tile([128, C], f32, name="bv_bc")
        bp_bc = kons.tile([128, C], f32, name="bp_bc")

        # ---------------- activations ----------------
        actx = ctx.enter_context(ExitStack())
        xk_p = actx.enter_context(tc.tile_pool(name="xk", bufs=8))
        xk = [xk_p.tile([128, S], bf16, name=f"xk{k}", tag="xk") for k in range(KC)]
        for n in range(2):  # halves so the first matmul chain starts early
            for k in range(KC):
                nc.sync.dma_start(
                    out=xk[k][:, ds(n * 512, 512)],
                    in_=xT[ts(k, 128), ds(n * 512, 512)],
                )
        # consts on the gpsimd queue (idle this early)
        nc.gpsimd.dma_start(out=cs_t[:], in_=cs[:])
        nc.gpsimd.dma_start(out=bqk_t[:], in_=bqk[:])
        nc.gpsimd.dma_start(out=bv_sb[:], in_=bv[:])
        nc.gpsimd.dma_start(out=bp_sb[:], in_=bp[:])
        nc.gpsimd.partition_broadcast(bv_bc[:], bv_sb[:])
        nc.gpsimd.partition_broadcast(bp_bc[:], bp_sb[:])

        wv_p = actx.enter_context(tc.tile_pool(name="wv", bufs=8))
        wv = [wv_p.tile([128, C], bf16, name=f"wv{k}", tag="wv") for k in range(KC)]
        wp_p = actx.enter_context(tc.tile_pool(name="wp", bufs=8))
        wp = [wp_p.tile([128, C], bf16, name=f"wp{k}", tag="wp") for k in range(KC)]

        wqk_p = actx.enter_context(tc.tile_pool(name="wqk", bufs=3))
        scr_p = actx.enter_context(tc.tile_pool(name="scr", bufs=2))
        tm_p = actx.enter_context(tc.tile_pool(name="tm", bufs=2))
        qkr_p = actx.enter_context(tc.tile_pool(name="qkr", bufs=4))
        vst_p = actx.enter_context(tc.tile_pool(name="vst", bufs=8))
        vst = [
            vst_p.tile([128, H * 65], bf16, name=f"vst{m}", tag="vst")
            for m in range(KC)
        ]
        pT_p = actx.enter_context(tc.tile_pool(name="pT", bufs=5))
        oraw_p = actx.enter_context(tc.tile_pool(name="oraw", bufs=2))
        rec_p = actx.enter_context(tc.tile_pool(name="rec", bufs=2))
        rcp_p = actx.enter_context(tc.tile_pool(name="rcp", bufs=2))
        rcplo_p = actx.enter_context(tc.tile_pool(name="rcplo", bufs=6))
        rb_p = actx.enter_context(tc.tile_pool(name="rb", bufs=8))
        aT_p = actx.enter_context(tc.tile_pool(name="aT", bufs=8))

        mm_ps = ctx.enter_context(tc.tile_pool(name="mm_ps", bufs=2, space="PSUM"))
        sc_ps = actx.enter_context(tc.tile_pool(name="sc_ps", bufs=2, space="PSUM"))
        oT_ps = actx.enter_context(tc.tile_pool(name="oT_ps", bufs=2, space="PSUM"))

        # paired W_qk loads: one DMA per (pair, a) -> (128, 8k x (2a x 128c))
        wqk_src = Wqk[0:C, :].rearrange("(k p) (a g c) -> p k g a c", p=128, a=2, g=8)

        def qk_pair_weights(pair):
            w = wqk_p.tile([128, 8 * 256], bf16, name=f"wqk{pair}", tag="wqk")
            wv4 = w[:].rearrange("p (k a c) -> p k a c", k=8, a=2)
            for a in range(2):
                nc.scalar.dma_start(
                    out=wv4[:, :, a, :], in_=wqk_src[:, :, pair, a, :]
                )
            return w

        # -------- fine-grained qk chunk emission (interleaved into attention)
        class QkChunk:
            """RoPE'd qkT chunk gm = a*8 + pair (a=0: q, a=1: k); emits its
            matmuls two at a time via step() so the PE queue stays mixed with
            attention work (in-order queues starve ACT otherwise)."""

            def __init__(self, pair, a, wts):
                self.gm = a * 8 + pair
                self.a = a
                self.wts = wts
                self.rr = scr_p.tile([128, 2 * S], bf16, name=f"rr{self.gm}", tag="rr")
                self.ps = [None, None]
                self.k = 0  # 0..15: (n, kc) pairs emitted

            def step(self, nmm=2):
                for _ in range(nmm):
                    if self.k >= 16:
                        return
                    n, kk = divmod(self.k, KC)
                    if kk == 0:
                        self.ps[n] = mm_ps.tile(
                            [128, 512], f32, name=f"qps{self.gm}_{n}", tag="mm"
                        )
                    nc.tensor.matmul(
                        self.ps[n][:],
                        self.wts[:, kk * 256 + self.a * 128 : kk * 256 + self.a * 128 + 128],
                        xk[kk][:, ds(n * 512, 512)],
                        start=(kk == 0),
                        stop=(kk == KC - 1),
                    )
                    self.k += 1
                    if kk == KC - 1:
                        nc.vector.tensor_scalar_add(
                            self.rr[:, ds(n * 512, 512)],
                            self.ps[n][:],
                            bqk_t[:, self.gm : self.gm + 1],
                        )

            def finish(self):
                self.step(16 - self.k)
                rr = self.rr
                # rotate-half copies ride the idle sync queue: descriptor
                # generation on the gpsimd queue would delay the broadcasts
                # behind it, head-of-line blocking the DVE normalize TTs
                for d0, s0 in ((0, 32), (32, 0), (64, 96), (96, 64)):
                    nc.sync.dma_start(
                        out=rr[d0 : d0 + 32, S : 2 * S], in_=rr[s0 : s0 + 32, 0:S]
                    )
                tm = tm_p.tile([128, 2 * S], bf16, name=f"tm{self.gm}", tag="tm")
                nc.vector.tensor_tensor(tm[:], rr[:], cs_t[:], MUL)
                qt = qkr_p.tile([128, S], bf16, name=f"qkr{self.gm}", tag="qkr")
                nc.vector.tensor_tensor(qt[:], tm[:, 0:S], tm[:, S : 2 * S], ADD)
                return qt

        # -------- v chunk: vst[mv] = (x @ Wv + bv)[mv*128:+128] with ones col
        def v_chunk(mv):
            t = vst[mv]
            ones_view = t[:, 0 : H * 65].rearrange("p (h u) -> p h u", u=65)[
                :, :, 64:65
            ]
            nc.vector.memset(ones_view, 1.0)
            for n in range(2):
                ps = mm_ps.tile([128, 512], f32, name=f"vps{mv}_{n}", tag="mm")
                for k in range(KC):
                    nc.tensor.matmul(
                        ps[:],
                        xk[k][:, ts(mv, 128)],
                        wv[k][:, ds(n * 512, 512)],
                        start=(k == 0),
                        stop=(k == KC - 1),
                    )
                ov = t[:, ds(65 * 8 * n, 65 * 8)].rearrange("p (h u) -> p h u", u=65)[
                    :, :, 0:64
                ]
                nc.vector.tensor_tensor(ov, ps[:], bv_bc[:, ds(n * 512, 512)], ADD)

        # -------- attention pair: (n, sk) steps, ACT-fused exp over both heads
        def attn_pair(pair, qt, kt, hooks=(None, None), prev_tail=None):
            """hooks[n]: callable(step) invoked once per sk step to interleave
            PE work (v chunks for pair 0, next pair's qk chunks otherwise).
            prev_tail: deferred normalize tail of the previous pair, emitted
            early in this pair's steps (keeps it off the pair boundary where
            the PE would stall waiting on the DVE queue)."""
            aT = aT_p.tile([128, S], bf16, name=f"aT{pair}", tag="aT")
            oraw = oraw_p.tile([128, S], bf16, name=f"oraw{pair}", tag="oraw")
            # denominator rows parked on partitions {0,32,64,96} (single-
            # partition APs must be 32-aligned); reciprocal runs on all 128
            # lanes at the same FD-bound cost, garbage lanes never read
            rec = rec_p.tile([128, 512], f32, name=f"rec{pair}", tag="rec")
            for n in range(2):
                pT = {}
                oT = [
                    oT_ps.tile([65, 512], f32, name=f"oT{pair}_{n}_{h}", tag="oT")
                    for h in range(2)
                ]

                def pv(sk):
                    for h in range(2):
                        nc.tensor.matmul(
                            oT[h][:],
                            vst[sk][:, 65 * (2 * pair + h) : 65 * (2 * pair + h) + 65],
                            pT[sk][:, ds(h * 512, 512)],
                            start=(sk == 0),
                            stop=(sk == KC - 1),
                        )

                for sk in range(KC):
                    scps = sc_ps.tile(
                        [128, 1024], f32, name=f"sc{pair}_{n}_{sk}", tag="sc"
                    )
                    for h in range(2):
                        nc.tensor.matmul(
                            scps[:, ds(h * 512, 512)],
                            kt[ds(h * 64, 64), ts(sk, 128)],
                            qt[ds(h * 64, 64), ds(n * 512, 512)],
                            start=True,
                            stop=True,
                        )
                    pT[sk] = pT_p.tile(
                        [128, 1024], bf16, name=f"pT{pair}_{n}_{sk}", tag="pT"
                    )
                    nc.scalar.activation(pT[sk][:], scps[:], AF.Exp, scale=0.125)
                    if sk >= 2:
                        pv(sk - 2)
                    if hooks[n] is not None:
                        hooks[n](sk)
                    if prev_tail is not None and n == 0 and sk == 1:
                        prev_tail()
                        prev_tail = None
                pv(KC - 2)
                pv(KC - 1)
                # free PSUM fast: denominator rows + raw o -> SBUF.  h0 rides
                # ACT, h1 rides DVE so the two banks drain in parallel (the
                # next n-phase's first PV waits on these).
                for h in range(2):
                    r = 32 * (2 * n + h)
                    if h == 0:
                        nc.scalar.activation(
                            rec[r : r + 1, :], oT[h][64:65, :], AF.Copy
                        )
                        nc.scalar.activation(
                            oraw[ds(h * 64, 64), ds(n * 512, 512)],
                            oT[h][0:64, :],
                            AF.Copy,
                        )
                    else:
                        nc.vector.tensor_copy(rec[r : r + 1, :], oT[h][64:65, :])
                        nc.vector.tensor_copy(
                            oraw[ds(h * 64, 64), ds(n * 512, 512)], oT[h][0:64, :]
                        )

            # deferred tail: batched reciprocal, staged broadcast sources
            # (partition_broadcast only reads physical partition 0), FULL-tile
            # broadcast writes (partial-tile writes raced with the normalize
            # multiply), then the normalize multiplies.
            def tail():
                rcp = rcp_p.tile([128, 512], bf16, name=f"rcp{pair}", tag="rcp")
                with nc.allow_low_precision(
                    reason="softmax denominators are O(1e2-1e3); bf16 "
                    "reciprocal adds ~0.4% rel err, within the 2e-2 gate"
                ):
                    nc.vector.reciprocal(rcp[:], rec[:])
                for n in range(2):
                    for h in range(2):
                        j = 2 * n + h
                        if j:
                            t = rcplo_p.tile(
                                [1, 512], bf16, name=f"rcp{pair}_{j}", tag="rcplo"
                            )
                            nc.sync.dma_start(out=t[:], in_=rcp[32 * j : 32 * j + 1, :])
                            src = t[0:1, :]
                        else:
                            src = rcp[0:1, :]
                        rb = rb_p.tile([128, 512], bf16, name=f"rb{pair}_{j}", tag="rb")
                        nc.gpsimd.partition_broadcast(rb[:], src)
                        nc.vector.tensor_tensor(
                            aT[ds(h * 64, 64), ds(n * 512, 512)],
                            oraw[ds(h * 64, 64), ds(n * 512, 512)],
                            rb[ds(h * 64, 64), :],
                            MUL,
                        )

            return aT, tail

        class QkFeeder:
            """Dispenses the next pair's q+k chunk matmuls in small quanta so
            they interleave into the current pair's attention steps (PE queues
            are in-order; a 32-MM block emitted at once would starve ACT)."""

            def __init__(self, pair, wts):
                self.chunks = [QkChunk(pair, 0, wts), QkChunk(pair, 1, wts)]
                self.done = []
                self.i = 0

            def feed(self, nmm):
                while nmm > 0 and self.i < 2:
                    ch = self.chunks[self.i]
                    take = min(nmm, 16 - ch.k)
                    ch.step(take)
                    nmm -= take
                    if ch.k >= 16:
                        self.done.append(ch.finish())
                        self.i += 1

            def result(self):
                self.feed(32)
                return self.done[0], self.done[1]  # qt, kt

        # ---------------- emission schedule ----------------
        # pair 0 qk first (early PE work while weights stream)
        f0 = QkFeeder(0, qk_pair_weights(0))
        qt, kt = f0.result()

        # Wv loads after pair-0 weights on the same queue
        for k in range(KC):
            nc.scalar.dma_start(out=wv[k][:], in_=Wv[ts(k, 128), :])

        aT_tiles = []
        feeder = QkFeeder(1, qk_pair_weights(1))
        pend_tail = None
        for pair in range(H // 2):
            if pair == 0:
                # v phase fills pair 0's n=0 steps; pair 1's qk fills n=1
                hooks = (
                    lambda step: v_chunk(step),
                    lambda step: feeder.feed(4),
                )
            elif pair < H // 2 - 1:
                hooks = (
                    lambda step: feeder.feed(3),
                    lambda step: feeder.feed(3),
                )
            else:
                hooks = (None, None)
            # mid-run Wp loads (sync queue, x is long since done)
            if pair == 4:
                for k in range(KC):
                    nc.sync.dma_start(out=wp[k][:], in_=Wp[ts(k, 128), :])
            aT, pend_tail = attn_pair(pair, qt, kt, hooks, prev_tail=pend_tail)
            aT_tiles.append(aT)
            if pair + 1 < H // 2:
                qt, kt = feeder.result()
                if pair + 2 < H // 2:
                    feeder = QkFeeder(pair + 2, qk_pair_weights(pair + 2))
        # ---------------- output projection ----------------
        # Pre-accumulate k=0..6 of the first six (m, n) groups BEFORE the last
        # pair's normalize tail: they depend only on aT[0..6], so the PE chews
        # them (and stays HAM-warm) while the tail drains on DVE+GPSIMD.
        # Groups 0-1 use the mm pool; 2-5 borrow score-PSUM tiles (free after
        # the last exp).
        ob_p = actx.enter_context(tc.tile_pool(name="ob", bufs=3))
        pre = {}
        sct = None
        for g in range(6):
            m, n = divmod(g, 2)
            if g < 2:
                pre[(m, n)] = mm_ps.tile([128, 512], f32, name=f"pp{m}_{n}", tag="mm")[
                    :
                ]
            else:
                if g % 2 == 0:
                    sct = sc_ps.tile([128, 1024], f32, name=f"ppsc{g}", tag="sc")
                pre[(m, n)] = sct[:, ds((g % 2) * 512, 512)]
            for k in range(KC - 1):
                nc.tensor.matmul(
                    pre[(m, n)],
                    aT_tiles[k][:, ts(m, 128)],
                    wp[k][:, ds(n * 512, 512)],
                    start=(k == 0),
                    stop=False,
                )
        pend_tail()  # last pair's normalize tail
        kl = KC - 1
        for (m, n), pp in pre.items():
            nc.tensor.matmul(
                pp,
                aT_tiles[kl][:, ts(m, 128)],
                wp[kl][:, ds(n * 512, 512)],
                start=False,
                stop=True,
            )
        for m in range(S // 128):
            ob = ob_p.tile([128, C], f32, name=f"ob{m}", tag="ob")
            for n in range(2):
                pp = pre.get((m, n))
                if pp is None:
                    ppt = mm_ps.tile([128, 512], f32, name=f"pp{m}_{n}", tag="mm")
                    for k in range(KC):
                        nc.tensor.matmul(
                            ppt[:],
                            aT_tiles[k][:, ts(m, 128)],
                            wp[k][:, ds(n * 512, 512)],
                            start=(k == 0),
                            stop=(k == KC - 1),
                        )
                    pp = ppt[:]
                nc.vector.tensor_tensor(
                    ob[:, ds(n * 512, 512)], pp, bp_bc[:, ds(n * 512, 512)], ADD
                )
            nc.sync.dma_start(out=out[ts(m, 128), :], in_=ob[:])

        actx.close()


def build_program():
    """Build + compile the Bass program (cached)."""
    if "nc" in _CACHE:
        return _CACHE["nc"]
    import concourse.tile as tile
    from concourse import bacc

    nc = bacc.Bacc(
        "TRN2", target_bir_lowering=False, debug=False, num_devices=N_CORES
    )
    with tile.TileContext(nc) as tc:
        _emit(tc)
    nc.compile()
    _CACHE["nc"] = nc
    return nc


def host_inputs(x, W_qkv, b_qkv, W_proj, b_proj):
    """Per-core input maps (host-side shard + layout prep)."""
    import ml_dtypes

    f = np.float32
    bf = ml_dtypes.bfloat16
    x = np.asarray(x, dtype=f)
    W_qkv = np.asarray(W_qkv, dtype=f)
    b_qkv = np.asarray(b_qkv, dtype=f)
    W_proj = np.asarray(W_proj, dtype=f)
    b_proj = np.asarray(b_proj, dtype=f)
    Wqk = np.ascontiguousarray(W_qkv[:, : 2 * C]).astype(bf)
    bqk = np.ascontiguousarray(b_qkv[: 2 * C].reshape(16, 128).T).astype(f)
    Wv = np.ascontiguousarray(W_qkv[:, 2 * C :]).astype(bf)
    bv = b_qkv[None, 2 * C :].astype(f)
    Wp = W_proj.astype(bf)
    bp = b_proj[None, :].astype(f)
    cs = _cs_table().astype(bf)
    maps = []
    for b in range(B):
        maps.append(
            {
                "xT": np.ascontiguousarray(x[b].T).astype(bf),
                "Wqk": Wqk,
                "bqk": bqk,
                "Wv": Wv,
                "bv": bv,
                "Wp": Wp,
                "bp": bp,
                "cs": cs,
            }
        )
    return maps


def _install_neff_cache():
    """Memoize the BIR->NEFF compile so repeat kernel() calls skip the
    multi-minute neuronxcc invocation (pure caching, same artifacts)."""
    if _CACHE.get("neff_cache"):
        return
    import hashlib
    import shutil
    import tempfile

    import concourse.bass2jax as b2j
    import concourse.bass_utils as bu

    cache_dir = os.path.join(tempfile.gettempdir(), "bass_neff_cache")
    os.makedirs(cache_dir, exist_ok=True)
    orig = bu.compile_bir_kernel

    def cached(bir_json, tmpdir, neff_name="file.neff"):
        raw = bir_json if isinstance(bir_json, bytes) else bir_json.encode()
        hit = os.path.join(cache_dir, hashlib.sha256(raw).hexdigest() + ".neff")
        if os.path.exists(hit):
            dst = os.path.join(tmpdir, neff_name)
            shutil.copyfile(hit, dst)
            return dst
        path = orig(bir_json, tmpdir, neff_name)
        try:
            shutil.copyfile(path, hit)
        except OSError:
            pass
        return path

    bu.compile_bir_kernel = cached
    b2j.compile_bir_kernel = cached
    _CACHE["neff_cache"] = True


def kernel(x, W_qkv, b_qkv, W_proj, b_proj):
    from concourse.bass_utils import run_bass_kernel_spmd

    _install_neff_cache()
    nc = build_program()
    in_maps = host_inputs(x, W_qkv, b_qkv, W_proj, b_proj)
    res = run_bass_kernel_spmd(nc, in_maps, list(range(N_CORES)))
    return np.stack([r["out"] for r in res.results], axis=0).astype(np.float32)


if __name__ == "__main__":
    nc = build_program()
    print("program built + compiled OK")
